# revision 5
# baseline (speedup 1.0000x reference)
"""Trainium2 Bass kernel for nn_Blur (upfirdn2d 4x4 blur, pad=(2,1)).

Formulation: out[i,j] = sum_{p,q} Kf[p,q] * x[i+p-2, j+q-2]   (Kf = flip(kernel2d))

For each W-tap q (4 taps), the H-convolution is a banded 64x64 matrix
Aq[i,h] = Kf[h-i+2, q].  Tolerance is 2e-2, so x streams as a single bf16
(the {1,3,9}/64 blur weights have <=4 mantissa bits: every bf16 product is
exact in fp32; end-to-end error ~4e-3).

W-taps are fused in PAIRS into the K=128 contraction: the host packs each
image with a LEADING ZERO column (65 cols/image) into partitions 0-63, and
one on-chip SBUF->SBUF DMA writes the same rows shifted left by one column
into partitions 64-127.  The zero columns land exactly where out-of-range
taps must contribute zero, so two matmuls per image cover all 4 taps with
no boundary fixups:
  pair(2,3): lhsT=[A2^T;A3^T], rhs cols c=1..64, out j=c-1 (start=True)
  pair(0,1): lhsT=[A0^T;A1^T], rhs cols c=0..62, out j=c+1 (accumulate)
Two such matmuls (8 images each, N~512) run CONCURRENTLY on disjoint PE
column groups (tile_position (0,0)/(0,64)), halving tensor time vs a
4-tap formulation.

The fp32 PSUM result is copied to SBUF as bf16 (engine alternates between
vector and scalar per batch), DMA'd back as [128,512] bf16 tiles, and
inverse-transposed + cast to f32 on the host.  HBM traffic per core is
8.55 MB in + 8.4 MB out -- half the f32 roofline.

Sharding: the 16*512 = 8192 independent (n,c) images are split into 8
contiguous slabs of 1024 images, one per NeuronCore (data-parallel).
"""

import ml_dtypes
import numpy as np

import concourse.bacc as bacc
import concourse.bass as bass
import concourse.mybir as mybir
import concourse.tile as tile
from concourse.bass_utils import run_bass_kernel_spmd

N_CORES = 8
IMG = 64                      # H = W
N_IMAGES = 16 * 512           # 8192
PER_CORE = N_IMAGES // N_CORES  # 1024
GROUP = 16                    # images per batch
N_BATCH = PER_CORE // GROUP   # 64
IMGW = 72                    # cols/image: zero col + 64 data + 7 zero pad
TILE_W = GROUP * IMGW         # 1152 (144B image stride: 16B-aligned)
DT = mybir.dt.float32
IN_DT = mybir.dt.bfloat16
NP_IN = ml_dtypes.bfloat16

LAST_RESULTS = None  # BassKernelResults of the most recent run (for test.py)


def _build_weights(kernel2d: np.ndarray) -> np.ndarray:
    """[128, 128] bf16 lhsT blocks: cols 0:64 = [A2^T;A3^T], 64:128 = [A0^T;A1^T].

    Aq[i,h] = Kf[h-i+2, q]; all entries in {1,3,9}/64 -- exact in bf16."""
    kf = np.flip(np.asarray(kernel2d, dtype=np.float64), (0, 1))
    a = np.zeros((4, IMG, IMG), dtype=np.float64)
    for q in range(4):
        for i in range(IMG):
            for p in range(4):
                h = i + p - 2
                if 0 <= h < IMG:
                    a[q, i, h] = kf[p, q]
    wts = np.zeros((128, 128), dtype=NP_IN)
    wts[:IMG, 0:IMG] = a[2].T.astype(NP_IN)
    wts[IMG:, 0:IMG] = a[3].T.astype(NP_IN)
    wts[:IMG, IMG:128] = a[0].T.astype(NP_IN)
    wts[IMG:, IMG:128] = a[1].T.astype(NP_IN)
    return wts


def _bass_module() -> bass.Bass:
    nc = bacc.Bacc(
        "TRN2",
        target_bir_lowering=False,
        debug=False,
        num_devices=N_CORES,
    )
    x_d = nc.dram_tensor("x", [N_BATCH, IMG, TILE_W], IN_DT, kind="ExternalInput")
    w_d = nc.dram_tensor("wts", [128, 128], IN_DT, kind="ExternalInput")
    o_d = nc.dram_tensor("out", [N_BATCH, 128, 512], IN_DT, kind="ExternalOutput")

    with tile.TileContext(nc) as tc:
        with (
            tc.tile_pool(name="const", bufs=1) as cpool,
            tc.tile_pool(name="inp", bufs=8) as ipool,
            tc.tile_pool(name="outp", bufs=8) as opool,
            tc.tile_pool(name="psum", bufs=8, space="PSUM") as ppool,
        ):
            w_tile = cpool.tile([128, 128], IN_DT)
            nc.sync.dma_start(w_tile[:], w_d[:])

            # HAM warmup: the PE clock-gate holds 1.2 GHz until ~3.4us of
            # sustained matmul activity.  Burn that window on dummy matmuls
            # (zeroed operands, result never read) that overlap the first
            # input DMA, so the real matmuls start at 2.4 GHz.
            dummy = cpool.tile([128, 512], IN_DT, tag="warm_sbuf")
            nc.gpsimd.memset(dummy[:], 0.0)
            warm_ps = ppool.tile([128, 512], DT, tag="ps")
            for _ in range(16):
                nc.tensor.matmul(
                    warm_ps[:], dummy[:, 0:128], dummy[:], start=True, stop=True
                )

            for b in range(N_BATCH):
                in_tile = ipool.tile([128, TILE_W], IN_DT)
                nc.sync.dma_start(in_tile[0:IMG, :], x_d[b])
                # shifted dup: partitions 64-127 = same rows, one col left
                # (through the hardware DMA queues; SBUF->SBUF, no HBM).
                # The per-image zero pad cols land exactly where
                # out-of-range taps must contribute zero.
                nc.scalar.dma_start(
                    in_tile[IMG:128, 0 : TILE_W - 1],
                    in_tile[0:IMG, 1:TILE_W],
                )
                rhs3 = in_tile[:, :].rearrange("p (g w) -> p g w", w=IMGW)
                ps = ppool.tile([128, 512], DT)
                for cg in range(2):
                    gs = slice(cg * 8, (cg + 1) * 8)
                    out3 = ps[cg * IMG : (cg + 1) * IMG, :].rearrange(
                        "p (g w) -> p g w", w=IMG
                    )
                    nc.tensor.matmul(
                        out3[:, :, 0:IMG],
                        w_tile[:, 0:IMG],
                        rhs3[:, gs, 1 : IMG + 1],
                        start=True,
                        stop=False,
                        tile_position=(0, cg * IMG),
                        skip_group_check=True,
                    )
                for cg in range(2):
                    gs = slice(cg * 8, (cg + 1) * 8)
                    out3 = ps[cg * IMG : (cg + 1) * IMG, :].rearrange(
                        "p (g w) -> p g w", w=IMG
                    )
                    nc.tensor.matmul(
                        out3[:, :, 1:IMG],
                        w_tile[:, IMG:128],
                        rhs3[:, gs, 0 : IMG - 1],
                        start=False,
                        stop=True,
                        tile_position=(0, cg * IMG),
                        skip_group_check=True,
                    )

                out_tile = opool.tile([128, 512], IN_DT)
                if b % 2 == 0:
                    nc.vector.tensor_copy(out_tile[:], ps[:])
                    nc.sync.dma_start(o_d[b], out_tile[:])
                else:
                    nc.scalar.copy(out_tile[:], ps[:])
                    nc.scalar.dma_start(o_d[b], out_tile[:])
    nc.compile()
    return nc


def _host_pack(x: np.ndarray) -> np.ndarray:
    """FULL x (8192,64,64) f32 -> [N_CORES, N_BATCH, 64, TILE_W] bf16.

    Partition dim = h; free dim = (g: 16 images, 72 cols: zero + 64 data
    + 7 zero pad; 144B stride keeps PE rhs fetches 16B-aligned)."""
    v = x.reshape(N_CORES, N_BATCH, GROUP, IMG, IMG).transpose(0, 1, 3, 2, 4)
    arr = np.zeros((N_CORES, N_BATCH, IMG, GROUP, IMGW), dtype=NP_IN)
    arr[..., 1 : IMG + 1] = v.astype(NP_IN)
    return arr.reshape(N_CORES, N_BATCH, IMG, TILE_W)


def _host_unpack(tiles: np.ndarray) -> np.ndarray:
    """[N_CORES, N_BATCH, 128, 512] bf16 -> (8192, 64, 64) f32.

    Partition dim = (cg, i); free dim = (g: 8, j); img = b*16 + cg*8 + g."""
    v = tiles.reshape(N_CORES, N_BATCH, 2, IMG, 8, IMG)
    v = v.transpose(0, 1, 2, 4, 3, 5)  # [core, b, cg, g, i, j]
    return v.reshape(N_IMAGES, IMG, IMG).astype(np.float32)


def kernel(x: np.ndarray, kernel: np.ndarray, _trace: bool = False) -> np.ndarray:
    global LAST_RESULTS
    x = np.ascontiguousarray(np.asarray(x, dtype=np.float32))
    n, c, h, w = x.shape
    assert (n, c, h, w) == (16, 512, 64, 64), x.shape

    shards = _host_pack(x.reshape(N_IMAGES, IMG, IMG))
    wts = _build_weights(kernel)
    in_maps = [{"x": shards[i], "wts": wts} for i in range(N_CORES)]

    nc = _bass_module()
    results = run_bass_kernel_spmd(
        nc, in_maps, core_ids=list(range(N_CORES)), trace=_trace
    )
    LAST_RESULTS = results

    tiles = np.stack([r["out"] for r in results.results])
    out = _host_unpack(tiles)
    return np.ascontiguousarray(out.reshape(n, c, h, w)).astype(np.float32)


# revision 12
# speedup vs baseline: 1.4128x; 1.4128x over previous
"""Trainium2 Bass kernel for nn_Blur (upfirdn2d 4x4 blur, pad=(2,1)).

Formulation: out[i,j] = sum_{p,q} Kf[p,q] * x[i+p-2, j+q-2]   (Kf = flip(kernel2d))

For each W-tap q (4 taps), the H-convolution is a banded 64x64 matrix
Aq[i,h] = Kf[h-i+2, q].  Tolerance is 2e-2, so x streams as a single bf16
(the {1,3,9}/64 blur weights have <=4 mantissa bits: every bf16 product is
exact in fp32; end-to-end error ~5e-3) -- HALF the HBM traffic of an
fp32-faithful hi/lo split.

The K=128 contraction is filled by stacking TWO images per partition set:
lhsT_q = blockdiag(Aq^T, Aq^T) [128,128], rhs = [x_even; x_odd] [128, N],
so each matmul computes both images' H-conv at M=128 (full PE width, no
tile_position games).  The 4 taps accumulate into one PSUM bank with
variable-width windows: tap q=2 covers the full width first (start=True
initializes the per-element has_written state everywhere), the narrower
boundary taps then accumulate into column subsets.  This keeps the PE
~90% busy, which holds the clock-boost (HAM) state -- schemes with less
tensor work demote the PE clock to 1.2 GHz and end up slower.

The fp32 PSUM result is copied to SBUF as bf16 (alternating vector /
scalar engines), DMA'd back as [128,512] bf16 tiles, and cast to f32 on
the host.  HBM per core: 8.4 MB in + 8.4 MB out = the ~47us roofline.

Sharding: the 16*512 = 8192 independent (n,c) images are split into 8
contiguous slabs of 1024 images, one per NeuronCore (data-parallel).
"""

import ml_dtypes
import numpy as np

import concourse.bacc as bacc
import concourse.bass as bass
import concourse.mybir as mybir
import concourse.tile as tile
from concourse.bass_utils import run_bass_kernel_spmd

N_CORES = 8
IMG = 64                      # H = W
N_IMAGES = 16 * 512           # 8192
PER_CORE = N_IMAGES // N_CORES  # 1024
GROUP = 16                    # images per batch (8 pairs stacked in K)
N_BATCH = PER_CORE // GROUP   # 64
TILE_W = 8 * IMG              # 512 free cols: 8 image pairs
# per-tap W windows: tap q reads x cols [XLO[q], +LEN[q]) and writes out
# cols [JLO[q], +LEN[q]).  q=2 goes first: full width, start=True.
TAP_ORDER = (2, 0, 1, 3)
XLO = (0, 0, 0, 1)
JLO = (2, 1, 0, 0)
LEN = (62, 63, 64, 63)
DT = mybir.dt.float32
IN_DT = mybir.dt.bfloat16
NP_IN = ml_dtypes.bfloat16

LAST_RESULTS = None  # BassKernelResults of the most recent run (for test.py)


def _build_weights(kernel2d: np.ndarray) -> np.ndarray:
    """[128, 512] bf16: cols [128q, 128q+128) = blockdiag(Aq^T, Aq^T)."""
    kf = np.flip(np.asarray(kernel2d, dtype=np.float64), (0, 1))
    wts = np.zeros((128, 512), dtype=NP_IN)
    for q in range(4):
        aq = np.zeros((IMG, IMG), dtype=np.float64)
        for i in range(IMG):
            for p in range(4):
                h = i + p - 2
                if 0 <= h < IMG:
                    aq[i, h] = kf[p, q]
        aqt = aq.T.astype(NP_IN)
        wts[:IMG, 128 * q : 128 * q + IMG] = aqt
        wts[IMG:, 128 * q + IMG : 128 * q + 128] = aqt
    return wts


def _bass_module() -> bass.Bass:
    nc = bacc.Bacc(
        "TRN2",
        target_bir_lowering=False,
        debug=False,
        num_devices=N_CORES,
    )
    x_d = nc.dram_tensor("x", [N_BATCH, 128, TILE_W], IN_DT, kind="ExternalInput")
    w_d = nc.dram_tensor("wts", [128, 512], IN_DT, kind="ExternalInput")
    o_d = nc.dram_tensor("out", [N_BATCH, 128, 512], IN_DT, kind="ExternalOutput")

    with tile.TileContext(nc) as tc:
        with (
            tc.tile_pool(name="const", bufs=1) as cpool,
            tc.tile_pool(name="inp", bufs=8) as ipool,
            tc.tile_pool(name="outp", bufs=8) as opool,
            tc.tile_pool(name="psum", bufs=8, space="PSUM") as ppool,
        ):
            w_tile = cpool.tile([128, 512], IN_DT)
            nc.sync.dma_start(w_tile[:], w_d[:])

            # HAM warmup: the PE clock-gate holds 1.2 GHz until ~3.4us of
            # sustained matmul activity.  Burn that window on dummy matmuls
            # (zeroed operands, result never read) that overlap the first
            # input DMA, so the real matmuls start at full clock.
            dummy = cpool.tile([128, 512], IN_DT, tag="warm_sbuf")
            nc.gpsimd.memset(dummy[:], 0.0)
            warm_ps = ppool.tile([128, 512], DT, tag="ps")
            for _ in range(16):
                nc.tensor.matmul(
                    warm_ps[:], dummy[:, 0:128], dummy[:], start=True, stop=True
                )

            for b in range(N_BATCH):
                in_tile = ipool.tile([128, TILE_W], IN_DT)
                nc.sync.dma_start(in_tile[:], x_d[b])
                rhs3 = in_tile[:].rearrange("p (g w) -> p g w", w=IMG)

                ps = ppool.tile([128, 512], DT)
                out3 = ps[:].rearrange("p (g w) -> p g w", w=IMG)
                for qi, q in enumerate(TAP_ORDER):
                    nc.tensor.matmul(
                        out3[:, :, JLO[q] : JLO[q] + LEN[q]],
                        w_tile[:, 128 * q : 128 * q + 128],
                        rhs3[:, :, XLO[q] : XLO[q] + LEN[q]],
                        start=(qi == 0),
                        stop=(qi == 3),
                    )

                out_tile = opool.tile([128, 512], IN_DT)
                if b % 2 == 0:
                    nc.vector.tensor_copy(out_tile[:], ps[:])
                    nc.sync.dma_start(o_d[b], out_tile[:])
                else:
                    nc.scalar.copy(out_tile[:], ps[:])
                    nc.scalar.dma_start(o_d[b], out_tile[:])
    nc.compile()
    return nc


def _host_pack(x: np.ndarray) -> np.ndarray:
    """FULL x (8192,64,64) f32 -> [N_CORES, N_BATCH, 128, 512] bf16.

    Partition dim = (a, h), a = image parity in pair; free dim = (g: 8
    pairs, w).  img = core*1024 + b*16 + g*2 + a."""
    v = x.reshape(N_CORES, N_BATCH, 8, 2, IMG, IMG).transpose(0, 1, 3, 4, 2, 5)
    return np.ascontiguousarray(v.astype(NP_IN)).reshape(
        N_CORES, N_BATCH, 128, TILE_W
    )


def _host_unpack(tiles: np.ndarray) -> np.ndarray:
    """[N_CORES, N_BATCH, 128, 512] bf16 -> (8192, 64, 64) f32."""
    v = tiles.reshape(N_CORES, N_BATCH, 2, IMG, 8, IMG)
    v = v.transpose(0, 1, 4, 2, 3, 5)  # [core, b, g, a, i, j]
    return v.reshape(N_IMAGES, IMG, IMG).astype(np.float32)


def kernel(x: np.ndarray, kernel: np.ndarray, _trace: bool = False) -> np.ndarray:
    global LAST_RESULTS
    x = np.ascontiguousarray(np.asarray(x, dtype=np.float32))
    n, c, h, w = x.shape
    assert (n, c, h, w) == (16, 512, 64, 64), x.shape

    shards = _host_pack(x.reshape(N_IMAGES, IMG, IMG))
    wts = _build_weights(kernel)
    in_maps = [{"x": shards[i], "wts": wts} for i in range(N_CORES)]

    nc = _bass_module()
    results = run_bass_kernel_spmd(
        nc, in_maps, core_ids=list(range(N_CORES)), trace=_trace
    )
    LAST_RESULTS = results

    tiles = np.stack([r["out"] for r in results.results])
    out = _host_unpack(tiles)
    return np.ascontiguousarray(out.reshape(n, c, h, w)).astype(np.float32)
